# revision 13
# baseline (speedup 1.0000x reference)
"""Trainium2 Bass kernel for nn_Bdfdv_51170240364850 (gnn_message_passing).

Computes, for mode pairs (il, im) with im <= il (L1 = 5 modes each way) and
spatial/velocity grid (nx=1024, nv=512):

  D[il,im] = base + (-1j)*im*bx*F[il,im] + cB*bm*F[il,im+1]
             + [im==0] Re(cC*bp*F[il,1])
  base     = 0.5*bm*F[il,im-1]  (il>=1, 1<=im<=il)   else  D0[il,im]

with bx = b[:,0], bm = b[:,1]+1j b[:,2], bp = conj(bm),
cB = -(il-im)(il+im+1)/2, cC = -il(il+1).

Strategy: pure data-parallel over nx across 8 NeuronCores (nx=128 per core on
the 128 SBUF partitions), fp16 I/O, and a three-engine split:

* PE (TensorEngine): every per-x product c(x)*T runs as a diagonal-weight
  matmul accumulating in PSUM (diag(c) @ tile scales partition row p by
  c(p)).  Mode-constant coefficient parts are folded into the operands:
  P = F[im-1] + 2cB*F[im+1] fuses the set & recurrence terms so each output
  column needs only THREE matmuls (0.5b1@P, -+0.5b2@Q, -+im b0@F).  The 12
  tiny diagonal weight tiles (0.5b1, +-0.5b2, +-m*b0, ones) ride in with the
  input DMA.
* DVE: fp16 4x tensor_scalar prescales (P/Q, G = 2cB0*F[1]) plus the im=0
  imaginary row as two fused scalar_tensor_tensor ops (D0i as the add
  operand).
* ACT: evacuates finished PSUM quads (one whole im-run per copy) into the
  fp16 output tile.

PSUM quads ([128, 4 banks] tiles, pool bufs=2) double-buffer the 8 banks;
outputs stream out one im-run at a time.  DMA (~40KB in + 28KB out per
partition, fp16) is the roofline.
"""

import numpy as np

import bass_rust
import concourse.bass as bass
import concourse.tile as tile
from concourse import mybir
from concourse.bass_utils import run_bass_kernel_spmd

L1 = 5
NX = 1024
NV = 512
NCORES = 8
XS = NX // NCORES  # 128, = SBUF partitions

F32 = mybir.dt.float32
F16 = mybir.dt.float16

# ---------------------------------------------------------------------------
# slot bookkeeping (im-major ordering of the 14 valid (im, il>=1) F/D slots)
S = [(im, il) for im in range(L1) for il in range(max(1, im), L1)]
SIDX = {s: k for k, s in enumerate(S)}
NS = len(S)                      # 14
RUN = {0: 0, 1: 4, 2: 8, 3: 11, 4: 13}   # start slot index of each im-run
RL = {0: 4, 1: 4, 2: 3, 3: 2, 4: 1}      # run lengths

CB_PAIRS = [(2, 1), (3, 1), (3, 2), (4, 1), (4, 2), (4, 3)]  # (il, im)

# F/D run-interleaved layout: run m holds [re slots | im slots] back-to-back,
# so each im-run moves as ONE contiguous DMA.
FOFF = {}
_o = 0
for _m in range(L1):
    FOFF[_m] = _o
    _o += 2 * RL[_m] * NV
assert _o == 2 * NS * NV


def _cB(il, im):
    return -(il - im) * (il + im + 1) / 2.0


# pin layout (fp16): [F runs (28 NV) | D0r (4) | D0i (4) | W diags (12x128)]
WOFF = 36 * NV
NDIAG = 12      # D1=0.5b1, D2=0.5b2, D3=-0.5b2, A+1..4=m*b0, A-1..4, ONES
DG_D1, DG_D2, DG_D3 = 0, 1, 2
DG_ONES = 11


def DG_AP(m):
    return 2 + m          # 3..6


def DG_AN(m):
    return 6 + m          # 7..10


CIN = WOFF + NDIAG * 128
# pscal (fp32): per-x scalars for the DVE im=0 imaginary chain
H1, H2 = 0, 1                    # 0.5*b1, 0.5*b2
NSCAL = 4
# pout layout (fp16): same run-interleaved layout as F
COUT = 2 * NS * NV


# ---------------------------------------------------------------------------
# The walrus build in this container rejects instructions carrying more than
# ONE sync-wait ("Too many sync wait commands", setupSyncWait in
# CoreV2/V3GenImpl). Tile's scheduler routinely attaches several. Post-pass:
# hoist all but the last wait of each instruction onto same-engine NOPs
# inserted immediately before it (same basic block, so per-engine program
# order is preserved).
def split_multiwaits(nc):
    for f in nc.m.functions:
        for blk in f.blocks:
            new = []
            changed = False
            for ins in blk.instructions:
                si = ins.sync_info
                if si is not None and len(si.on_wait) > 1:
                    waits = list(si.on_wait)
                    for w in waits[:-1]:
                        nop = mybir.InstNoOp(
                            name=nc.get_next_instruction_name(),
                            engine=ins.engine,
                            bass_nofuse=True,
                            sync_info=mybir.SyncInfo(on_wait=[w],
                                                     on_update=[]),
                        )
                        new.append(nop)
                    ins.sync_info = bass_rust.SyncInfo(
                        on_wait=[waits[-1]], on_update=list(si.on_update))
                    changed = True
                new.append(ins)
            if changed:
                blk.instructions = new


# ---------------------------------------------------------------------------
def _pair(ap, step_elems, nblocks=2):
    """Turn a contiguous [P, L] AP into [P, nblocks, L] with the given
    element step between blocks."""
    c = ap.copy()
    v = c.ap
    last = v.pop()
    v.append((step_elems, nblocks))
    v.append(tuple(last))
    c.ap = v
    return c


def build_bass(split=True):
    MULT = mybir.AluOpType.mult
    ADD = mybir.AluOpType.add

    nc = bass.Bass()
    pin = nc.dram_tensor("pin", [XS, CIN], F16, kind="ExternalInput").ap()
    pscal = nc.dram_tensor("pscal", [XS, NSCAL], F32,
                           kind="ExternalInput").ap()
    pout = nc.dram_tensor("pout", [XS, COUT], F16, kind="ExternalOutput").ap()

    with tile.TileContext(nc) as tc:
        with tc.tile_pool(name="m", bufs=1) as pool, \
             tc.psum_pool(name="p", bufs=2) as ppool:
            fF = pool.tile([XS, 2 * NS * NV], F16, tag="fF")
            fD0 = pool.tile([XS, 8 * NV], F16, tag="fD0")
            fW = pool.tile([XS, NDIAG * 128], F16, tag="fW")
            scal = pool.tile([XS, NSCAL], F32, tag="scal")
            P = pool.tile([XS, 2 * 6 * NV], F16, tag="P")
            G = pool.tile([XS, 2 * 4 * NV], F16, tag="G")
            G2 = pool.tile([XS, 2 * 4 * NV], F16, tag="G2")
            OUT = pool.tile([XS, 2 * NS * NV], F16, tag="OUT")

            def fslot(k, imag, n=1):
                m = S[k][0]
                o = FOFF[m] + (imag * RL[m] + (k - RUN[m])) * NV
                return fF[:, o:o + n * NV]

            def fr(k):
                return fslot(k, 0)

            def fi(k):
                return fslot(k, 1)

            def pr(j):
                return P[:, j * NV:(j + 1) * NV]

            def pi(j):
                return P[:, (6 + j) * NV:(7 + j) * NV]

            def W(j):
                return fW[:, j * 128:(j + 1) * 128]

            def outr(k, n=1):
                m = S[k][0]
                o = FOFF[m] + (k - RUN[m]) * NV
                return OUT[:, o:o + n * NV]

            def outi(k, n=1):
                m = S[k][0]
                o = FOFF[m] + (RL[m] + k - RUN[m]) * NV
                return OUT[:, o:o + n * NV]

            def sc(col):
                return scal[:, col:col + 1]

            # ---- input DMAs: back-to-back issue; queue FIFO order makes
            # emission order the arrival priority without per-link latency.
            nc.sync.dma_start(scal[:], pscal[:])
            nc.sync.dma_start(fW[:], pin[:, WOFF:WOFF + NDIAG * 128])

            def in_run(m):
                o = FOFF[m]
                n = 2 * RL[m] * NV
                nc.sync.dma_start(fF[:, o:o + n], pin[:, o:o + n])

            in_run(0)
            in_run(1)
            in_run(2)
            nc.sync.dma_start(fD0[:], pin[:, 28 * NV:36 * NV])
            in_run(3)
            in_run(4)

            # ---- DVE prescales ----
            def presc_G(il):        # (Gr,Gi) = 2*cB0(il) * (Fr1,Fi1)
                k1 = SIDX[(1, il)]
                nc.vector.tensor_scalar_mul(
                    _pair(G[:, (il - 1) * NV:il * NV], 4 * NV),
                    _pair(fr(k1), RL[1] * NV),
                    float(-il * (il + 1)))

            def presc_G2():         # G'r = 3*Gr ; G'i = -Gi
                nc.vector.tensor_scalar_mul(
                    G2[:, 0:4 * NV], G[:, 0:4 * NV], 3.0)
                nc.vector.tensor_scalar_mul(
                    G2[:, 4 * NV:8 * NV], G[:, 4 * NV:8 * NV], -1.0)

            def presc_P(j):         # (Pr,Pi) = F[im-1] + 2cB * F[im+1]
                il, im = CB_PAIRS[j]
                ks = SIDX[(im + 1, il)]
                kb = SIDX[(im - 1, il)]
                nc.vector.scalar_tensor_tensor(
                    _pair(pr(j), 6 * NV),
                    _pair(fr(ks), RL[im + 1] * NV),
                    2.0 * _cB(il, im),
                    _pair(fr(kb), RL[im - 1] * NV),
                    MULT, ADD)

            # DVE program order = emission order; interleave the im=0
            # imaginary-row chain into the gaps left by input-run arrival.
            gr = G[:, 0:4 * NV]
            gi = G[:, 4 * NV:8 * NV]
            d0i = fD0[:, 4 * NV:8 * NV]
            for il in range(1, L1):
                presc_G(il)
            presc_G2()
            for j in (0, 1, 3):     # group-1 operands (need runs 0 & 2)
                presc_P(j)
            nc.vector.scalar_tensor_tensor(   # Di0 = D0i + 0.5b1*Gi ...
                outi(0, 4), gi, sc(H1), d0i, MULT, ADD)
            for j in (2, 4):        # group-2 (runs 1 & 3)
                presc_P(j)
            presc_P(5)              # group-3 (runs 2 & 4)
            nc.vector.scalar_tensor_tensor(   # ... + 0.5b2*Gr
                outi(0, 4), gr, sc(H2), outi(0, 4), MULT, ADD)

            # ---- PE groups: quad PSUM tiles, three matmuls per column.
            # diag slot (il==g) reads F directly; middle slots read the
            # merged P/Q operands (set + recurrence fused by the prescale).
            evac = []               # (quad, psum_cols, OUT offset) FIFO

            def mm(quad, lo, j, rhs, start=False, stop=False):
                nc.tensor.matmul(quad[:, lo * NV:(lo + 1) * NV], W(j), rhs,
                                 start=start, stop=stop,
                                 skip_group_check=True)

            def group(g):
                n = RL[g]
                qR = ppool.tile([XS, 4 * NV], F32, tag="q", name=f"qR{g}")
                qI = ppool.tile([XS, 4 * NV], F32, tag="q", name=f"qI{g}")
                ils = list(range(g, L1))
                kp = SIDX[(g - 1, g)]
                ks = {il: SIDX[(g, il)] for il in ils}
                jj = {il: CB_PAIRS.index((il, g)) for il in ils if il > g}
                # diag slot: direct F sources (earliest-arriving runs)
                mm(qR, 0, DG_D1, fr(kp), start=True)
                mm(qI, 0, DG_D1, fi(kp), start=True)
                mm(qR, 0, DG_D3, fi(kp))
                mm(qI, 0, DG_D2, fr(kp))
                # cA pass opens the middle banks, closes the diag bank
                for il in ils:
                    mm(qR, il - g, DG_AP(g), fi(ks[il]), start=(il > g),
                       stop=(il == g))
                for il in ils:
                    mm(qI, il - g, DG_AN(g), fr(ks[il]), start=(il > g),
                       stop=(il == g))
                # merged set+cB products close the middle banks
                for il in ils:
                    if il > g:
                        mm(qR, il - g, DG_D1, pr(jj[il]))
                        mm(qI, il - g, DG_D1, pi(jj[il]))
                for il in ils:
                    if il > g:
                        mm(qR, il - g, DG_D3, pi(jj[il]), stop=True)
                        mm(qI, il - g, DG_D2, pr(jj[il]), stop=True)
                evac.append((qR, n, FOFF[g]))
                evac.append((qI, n, FOFF[g] + n * NV))

            def flush_evac():
                while evac:
                    q, n, o = evac.pop(0)
                    nc.scalar.copy(OUT[:, o:o + n * NV], q[:, 0:n * NV])

            def out_run(m):
                o = FOFF[m]
                n = 2 * RL[m] * NV
                nc.sync.dma_start(pout[:, o:o + n], OUT[:, o:o + n])

            group(1)
            flush_evac()
            group(2)
            out_run(1)
            flush_evac()

            # im0 real row on PE: Dr0 = 0.5b1@G'r - 0.5b2@G'i + ones@D0r
            q0 = ppool.tile([XS, 4 * NV], F32, tag="q", name="q0")
            for il in range(1, L1):
                mm(q0, il - 1, DG_D1, G2[:, (il - 1) * NV:il * NV],
                   start=True)
            for il in range(1, L1):
                mm(q0, il - 1, DG_D3, G2[:, (3 + il) * NV:(4 + il) * NV])
            for il in range(1, L1):
                mm(q0, il - 1, DG_ONES, fD0[:, (il - 1) * NV:il * NV],
                   stop=True)
            out_run(2)

            group(3)
            evac.insert(0, (q0, 4, FOFF[0]))
            flush_evac()
            group(4)
            out_run(3)
            flush_evac()
            out_run(0)
            out_run(4)

    if split:
        split_multiwaits(nc)
    return nc


# ---------------------------------------------------------------------------
def pack_inputs(prev_f_re, prev_f_im, delta0_re, delta0_im, b):
    """-> list of per-core {'pin': [XS, CIN] f16, 'pscal': [XS, 4] f32}."""
    pr = np.asarray(prev_f_re, np.float32)
    pi = np.asarray(prev_f_im, np.float32)
    d0r = np.asarray(delta0_re, np.float32)
    d0i = np.asarray(delta0_im, np.float32)
    bb = np.asarray(b, np.float32)
    ar = np.arange(XS)
    in_maps = []
    for c in range(NCORES):
        X = slice(c * XS, (c + 1) * XS)
        p = np.zeros((XS, CIN), np.float16)
        for k, (im, il) in enumerate(S):
            o = FOFF[im] + (k - RUN[im]) * NV
            p[:, o:o + NV] = pr[il, im, X, :]
            o += RL[im] * NV
            p[:, o:o + NV] = pi[il, im, X, :]
        for il in range(1, L1):
            p[:, (28 + il - 1) * NV:(29 + il - 1) * NV] = d0r[il, 0, X, :]
            p[:, (32 + il - 1) * NV:(33 + il - 1) * NV] = d0i[il, 0, X, :]
        b0, b1, b2 = bb[X, 0], bb[X, 1], bb[X, 2]
        diags = [0.5 * b1, 0.5 * b2, -0.5 * b2,
                 1.0 * b0, 2.0 * b0, 3.0 * b0, 4.0 * b0,
                 -1.0 * b0, -2.0 * b0, -3.0 * b0, -4.0 * b0,
                 np.ones(XS, np.float32)]
        for j, cx in enumerate(diags):
            blk = np.zeros((XS, 128), np.float16)
            blk[ar, ar] = cx.astype(np.float16)
            p[:, WOFF + j * 128:WOFF + (j + 1) * 128] = blk
        ps = np.zeros((XS, NSCAL), np.float32)
        ps[:, H1] = 0.5 * b1
        ps[:, H2] = 0.5 * b2
        in_maps.append({"pin": p, "pscal": ps})
    return in_maps


def unpack_outputs(results, delta0_re, delta0_im):
    out = np.zeros((L1, L1, NX, NV), np.complex64)
    out[0, 0] = np.asarray(delta0_re[0, 0]) + 1j * np.asarray(delta0_im[0, 0])
    for c in range(NCORES):
        X = slice(c * XS, (c + 1) * XS)
        p = results[c]["pout"]
        for k, (im, il) in enumerate(S):
            o = FOFF[im] + (k - RUN[im]) * NV
            dr = p[:, o:o + NV].astype(np.float32)
            o += RL[im] * NV
            di = p[:, o:o + NV].astype(np.float32)
            out[il, im, X, :] = dr + 1j * di
    return out


_NC_CACHE = None


def get_nc():
    global _NC_CACHE
    if _NC_CACHE is None:
        _NC_CACHE = build_bass()
    return _NC_CACHE


def kernel(prev_f_re, prev_f_im, delta0_re, delta0_im, b, v):
    in_maps = pack_inputs(prev_f_re, prev_f_im, delta0_re, delta0_im, b)
    res = run_bass_kernel_spmd(get_nc(), in_maps, list(range(NCORES)))
    return unpack_outputs(res.results, delta0_re, delta0_im)


# revision 14
# speedup vs baseline: 1.1369x; 1.1369x over previous
"""Trainium2 Bass kernel for nn_Bdfdv_51170240364850 (gnn_message_passing).

Computes, for mode pairs (il, im) with im <= il (L1 = 5 modes each way) and
spatial/velocity grid (nx=1024, nv=512):

  D[il,im] = base + (-1j)*im*bx*F[il,im] + cB*bm*F[il,im+1]
             + [im==0] Re(cC*bp*F[il,1])
  base     = 0.5*bm*F[il,im-1]  (il>=1, 1<=im<=il)   else  D0[il,im]

with bx = b[:,0], bm = b[:,1]+1j b[:,2], bp = conj(bm),
cB = -(il-im)(il+im+1)/2, cC = -il(il+1).

Strategy: pure data-parallel over nx across 8 NeuronCores (nx=128 per core on
the 128 SBUF partitions), fp16 I/O, and a three-engine split:

* PE (TensorEngine): every per-x product c(x)*T runs as a diagonal-weight
  matmul accumulating in PSUM (diag(c) @ tile scales partition row p by
  c(p)).  Mode-constant coefficient parts are folded into the operands:
  P = F[im-1] + 2cB*F[im+1] fuses the set & recurrence terms so each output
  column needs only THREE matmuls (0.5b1@P, -+0.5b2@Q, -+im b0@F).  The 12
  tiny diagonal weight tiles (0.5b1, +-0.5b2, +-m*b0, ones) ride in with the
  input DMA.
* DVE: fp16 4x tensor_scalar prescales (P/Q, G = 2cB0*F[1]) plus the im=0
  imaginary row as two fused scalar_tensor_tensor ops (D0i as the add
  operand).
* ACT: evacuates finished PSUM quads (one whole im-run per copy) into the
  fp16 output tile.

PSUM quads ([128, 4 banks] tiles, pool bufs=2) double-buffer the 8 banks;
outputs stream out one im-run at a time.  DMA (~40KB in + 28KB out per
partition, fp16) is the roofline.
"""

import numpy as np

import bass_rust
import concourse.bass as bass
import concourse.tile as tile
from concourse import mybir
from concourse.bass_utils import run_bass_kernel_spmd

L1 = 5
NX = 1024
NV = 512
NCORES = 8
XS = NX // NCORES  # 128, = SBUF partitions

F32 = mybir.dt.float32
F16 = mybir.dt.float16

# ---------------------------------------------------------------------------
# slot bookkeeping (im-major ordering of the 14 valid (im, il>=1) F/D slots)
S = [(im, il) for im in range(L1) for il in range(max(1, im), L1)]
SIDX = {s: k for k, s in enumerate(S)}
NS = len(S)                      # 14
RUN = {0: 0, 1: 4, 2: 8, 3: 11, 4: 13}   # start slot index of each im-run
RL = {0: 4, 1: 4, 2: 3, 3: 2, 4: 1}      # run lengths

CB_PAIRS = [(2, 1), (3, 1), (3, 2), (4, 1), (4, 2), (4, 3)]  # (il, im)

# F/D run-interleaved layout: run m holds [re slots | im slots] back-to-back,
# so each im-run moves as ONE contiguous DMA.
FOFF = {}
_o = 0
for _m in range(L1):
    FOFF[_m] = _o
    _o += 2 * RL[_m] * NV
assert _o == 2 * NS * NV


def _cB(il, im):
    return -(il - im) * (il + im + 1) / 2.0


# pin layout (fp16): [F runs (28 NV) | D0r (4) | D0i (4) | W diags (12x128)]
WOFF = 36 * NV
NDIAG = 13      # 0.5b1, +-0.5b2, A+1..4=m*b0, A-1..4, ones, 1.5b1
DG_D1, DG_D2, DG_D3 = 0, 1, 2
DG_ONES = 11
DG_D6 = 12


def DG_AP(m):
    return 2 + m          # 3..6


def DG_AN(m):
    return 6 + m          # 7..10


CIN = WOFF + NDIAG * 128
# pscal (fp32): per-x scalars for the DVE im=0 imaginary chain
H1, H2 = 0, 1                    # 0.5*b1, 0.5*b2
NSCAL = 4
# pout layout (fp16): same run-interleaved layout as F
COUT = 2 * NS * NV


# ---------------------------------------------------------------------------
# The walrus build in this container rejects instructions carrying more than
# ONE sync-wait ("Too many sync wait commands", setupSyncWait in
# CoreV2/V3GenImpl). Tile's scheduler routinely attaches several. Post-pass:
# hoist all but the last wait of each instruction onto same-engine NOPs
# inserted immediately before it (same basic block, so per-engine program
# order is preserved).
def split_multiwaits(nc):
    for f in nc.m.functions:
        for blk in f.blocks:
            new = []
            changed = False
            for ins in blk.instructions:
                si = ins.sync_info
                if si is not None and len(si.on_wait) > 1:
                    waits = list(si.on_wait)
                    for w in waits[:-1]:
                        nop = mybir.InstNoOp(
                            name=nc.get_next_instruction_name(),
                            engine=ins.engine,
                            bass_nofuse=True,
                            sync_info=mybir.SyncInfo(on_wait=[w],
                                                     on_update=[]),
                        )
                        new.append(nop)
                    ins.sync_info = bass_rust.SyncInfo(
                        on_wait=[waits[-1]], on_update=list(si.on_update))
                    changed = True
                new.append(ins)
            if changed:
                blk.instructions = new


# ---------------------------------------------------------------------------
def _pair(ap, step_elems, nblocks=2):
    """Turn a contiguous [P, L] AP into [P, nblocks, L] with the given
    element step between blocks."""
    c = ap.copy()
    v = c.ap
    last = v.pop()
    v.append((step_elems, nblocks))
    v.append(tuple(last))
    c.ap = v
    return c


def build_bass(split=True):
    MULT = mybir.AluOpType.mult
    ADD = mybir.AluOpType.add

    nc = bass.Bass()
    pin = nc.dram_tensor("pin", [XS, CIN], F16, kind="ExternalInput").ap()
    pscal = nc.dram_tensor("pscal", [XS, NSCAL], F32,
                           kind="ExternalInput").ap()
    pout = nc.dram_tensor("pout", [XS, COUT], F16, kind="ExternalOutput").ap()

    with tile.TileContext(nc) as tc:
        with tc.tile_pool(name="m", bufs=1) as pool, \
             tc.psum_pool(name="p", bufs=8) as ppool:
            fF = pool.tile([XS, 2 * NS * NV], F16, tag="fF")
            fD0 = pool.tile([XS, 8 * NV], F16, tag="fD0")
            fW = pool.tile([XS, NDIAG * 128], F16, tag="fW")
            scal = pool.tile([XS, NSCAL], F32, tag="scal")
            P = pool.tile([XS, 2 * 6 * NV], F16, tag="P")
            G = pool.tile([XS, 2 * 4 * NV], F16, tag="G")
            OUT = pool.tile([XS, 2 * NS * NV], F16, tag="OUT")

            def fslot(k, imag, n=1):
                m = S[k][0]
                o = FOFF[m] + (imag * RL[m] + (k - RUN[m])) * NV
                return fF[:, o:o + n * NV]

            def fr(k):
                return fslot(k, 0)

            def fi(k):
                return fslot(k, 1)

            def pr(j):
                return P[:, j * NV:(j + 1) * NV]

            def pi(j):
                return P[:, (6 + j) * NV:(7 + j) * NV]

            def W(j):
                return fW[:, j * 128:(j + 1) * 128]

            def outr(k, n=1):
                m = S[k][0]
                o = FOFF[m] + (k - RUN[m]) * NV
                return OUT[:, o:o + n * NV]

            def outi(k, n=1):
                m = S[k][0]
                o = FOFF[m] + (RL[m] + k - RUN[m]) * NV
                return OUT[:, o:o + n * NV]

            def sc(col):
                return scal[:, col:col + 1]

            # ---- input DMAs: back-to-back issue; queue FIFO order makes
            # emission order the arrival priority without per-link latency.
            nc.sync.dma_start(scal[:], pscal[:])
            nc.sync.dma_start(fW[:], pin[:, WOFF:WOFF + NDIAG * 128])

            def in_run(m):
                o = FOFF[m]
                n = 2 * RL[m] * NV
                nc.sync.dma_start(fF[:, o:o + n], pin[:, o:o + n])

            in_run(0)
            in_run(1)
            in_run(2)
            nc.sync.dma_start(fD0[:], pin[:, 28 * NV:36 * NV])
            in_run(3)
            in_run(4)

            # ---- DVE prescales ----
            def presc_G(il):        # (Gr,Gi) = 2*cB0(il) * (Fr1,Fi1)
                k1 = SIDX[(1, il)]
                nc.vector.tensor_scalar_mul(
                    _pair(G[:, (il - 1) * NV:il * NV], 4 * NV),
                    _pair(fr(k1), RL[1] * NV),
                    float(-il * (il + 1)))

            def presc_SF(j):        # P = 2cB * F[im+1]   (fp16 TS at 4x)
                il, im = CB_PAIRS[j]
                ks = SIDX[(im + 1, il)]
                nc.vector.tensor_scalar_mul(
                    _pair(pr(j), 6 * NV),
                    _pair(fr(ks), RL[im + 1] * NV),
                    2.0 * _cB(il, im))

            def presc_P(j):         # P += F[im-1]          (fp16 TT at 2x)
                il, im = CB_PAIRS[j]
                kb = SIDX[(im - 1, il)]
                nc.vector.tensor_tensor(
                    _pair(pr(j), 6 * NV),
                    _pair(pr(j), 6 * NV),
                    _pair(fr(kb), RL[im - 1] * NV),
                    ADD)

            # DVE program order = emission order; interleave the im=0
            # imaginary-row chain into the gaps left by input-run arrival.
            gr = G[:, 0:4 * NV]
            gi = G[:, 4 * NV:8 * NV]
            d0i = fD0[:, 4 * NV:8 * NV]
            for j in (0, 1, 3):     # group-1 operands (need runs 0 & 2)
                presc_SF(j)
                presc_P(j)
            for il in range(1, L1):
                presc_G(il)
            for j in (2, 4):        # group-2 (runs 1 & 3)
                presc_SF(j)
                presc_P(j)
            nc.vector.scalar_tensor_tensor(   # Di0 = D0i + 0.5b1*Gi ...
                outi(0, 4), gi, sc(H1), d0i, MULT, ADD)
            presc_SF(5)             # group-3 (runs 2 & 4)
            presc_P(5)
            nc.vector.scalar_tensor_tensor(   # ... + 0.5b2*Gr
                outi(0, 4), gr, sc(H2), outi(0, 4), MULT, ADD)

            # ---- PE groups: quad PSUM tiles, three matmuls per column.
            # diag slot (il==g) reads F directly; middle slots read the
            # merged P/Q operands (set + recurrence fused by the prescale).
            evac = []               # (quad, psum_cols, OUT offset) FIFO

            def mm(bank, j, rhs, start=False, stop=False):
                nc.tensor.matmul(bank[:], W(j), rhs, start=start, stop=stop,
                                 skip_group_check=True)

            def group(g):
                n = RL[g]
                ils = list(range(g, L1))
                bR = {}
                bI = {}
                for il in ils:
                    bR[il] = ppool.tile([XS, NV], F32, tag="bk",
                                        name=f"bR{g}_{il}")
                    bI[il] = ppool.tile([XS, NV], F32, tag="bk",
                                        name=f"bI{g}_{il}")
                kp = SIDX[(g - 1, g)]
                ks = {il: SIDX[(g, il)] for il in ils}
                jj = {il: CB_PAIRS.index((il, g)) for il in ils if il > g}
                # diag slot: direct F sources (earliest-arriving runs)
                mm(bR[g], DG_D1, fr(kp), start=True)
                mm(bI[g], DG_D1, fi(kp), start=True)
                mm(bR[g], DG_D3, fi(kp))
                mm(bI[g], DG_D2, fr(kp))
                # cA pass opens the middle banks, closes the diag bank
                for il in ils:
                    mm(bR[il], DG_AP(g), fi(ks[il]), start=(il > g),
                       stop=(il == g))
                for il in ils:
                    mm(bI[il], DG_AN(g), fr(ks[il]), start=(il > g),
                       stop=(il == g))
                # merged set+cB products close the middle banks
                for il in ils:
                    if il > g:
                        mm(bR[il], DG_D1, pr(jj[il]))
                        mm(bI[il], DG_D1, pi(jj[il]))
                for il in ils:
                    if il > g:
                        mm(bR[il], DG_D3, pi(jj[il]), stop=True)
                        mm(bI[il], DG_D2, pr(jj[il]), stop=True)
                for il in ils:
                    evac.append((bR[il], outr(ks[il])))
                    evac.append((bI[il], outi(ks[il])))

            def flush_evac():
                while evac:
                    q, dst = evac.pop(0)
                    nc.scalar.copy(dst, q[:])

            def out_run(m):
                o = FOFF[m]
                n = 2 * RL[m] * NV
                nc.sync.dma_start(pout[:, o:o + n], OUT[:, o:o + n])

            group(1)
            flush_evac()
            group(2)
            out_run(1)
            flush_evac()

            # im0 real row on PE: Dr0 = 1.5b1@Gr + 0.5b2@Gi + ones@D0r
            b0k = {}
            for il in range(1, L1):
                b0k[il] = ppool.tile([XS, NV], F32, tag="bk",
                                     name=f"b0_{il}")
            for il in range(1, L1):
                mm(b0k[il], DG_D6, G[:, (il - 1) * NV:il * NV], start=True)
            for il in range(1, L1):
                mm(b0k[il], DG_D2, G[:, (3 + il) * NV:(4 + il) * NV])
            for il in range(1, L1):
                mm(b0k[il], DG_ONES, fD0[:, (il - 1) * NV:il * NV],
                   stop=True)
            out_run(2)

            group(3)
            for il in range(1, L1):
                evac.insert(il - 1, (b0k[il], outr(il - 1)))
            flush_evac()
            group(4)
            out_run(3)
            flush_evac()
            out_run(0)
            out_run(4)

    if split:
        split_multiwaits(nc)
    return nc


# ---------------------------------------------------------------------------
def pack_inputs(prev_f_re, prev_f_im, delta0_re, delta0_im, b):
    """-> list of per-core {'pin': [XS, CIN] f16, 'pscal': [XS, 4] f32}."""
    pr = np.asarray(prev_f_re, np.float32)
    pi = np.asarray(prev_f_im, np.float32)
    d0r = np.asarray(delta0_re, np.float32)
    d0i = np.asarray(delta0_im, np.float32)
    bb = np.asarray(b, np.float32)
    ar = np.arange(XS)
    in_maps = []
    for c in range(NCORES):
        X = slice(c * XS, (c + 1) * XS)
        p = np.zeros((XS, CIN), np.float16)
        for k, (im, il) in enumerate(S):
            o = FOFF[im] + (k - RUN[im]) * NV
            p[:, o:o + NV] = pr[il, im, X, :]
            o += RL[im] * NV
            p[:, o:o + NV] = pi[il, im, X, :]
        for il in range(1, L1):
            p[:, (28 + il - 1) * NV:(29 + il - 1) * NV] = d0r[il, 0, X, :]
            p[:, (32 + il - 1) * NV:(33 + il - 1) * NV] = d0i[il, 0, X, :]
        b0, b1, b2 = bb[X, 0], bb[X, 1], bb[X, 2]
        diags = [0.5 * b1, 0.5 * b2, -0.5 * b2,
                 1.0 * b0, 2.0 * b0, 3.0 * b0, 4.0 * b0,
                 -1.0 * b0, -2.0 * b0, -3.0 * b0, -4.0 * b0,
                 np.ones(XS, np.float32), 1.5 * b1]
        for j, cx in enumerate(diags):
            blk = np.zeros((XS, 128), np.float16)
            blk[ar, ar] = cx.astype(np.float16)
            p[:, WOFF + j * 128:WOFF + (j + 1) * 128] = blk
        ps = np.zeros((XS, NSCAL), np.float32)
        ps[:, H1] = 0.5 * b1
        ps[:, H2] = 0.5 * b2
        in_maps.append({"pin": p, "pscal": ps})
    return in_maps


def unpack_outputs(results, delta0_re, delta0_im):
    out = np.zeros((L1, L1, NX, NV), np.complex64)
    out[0, 0] = np.asarray(delta0_re[0, 0]) + 1j * np.asarray(delta0_im[0, 0])
    for c in range(NCORES):
        X = slice(c * XS, (c + 1) * XS)
        p = results[c]["pout"]
        for k, (im, il) in enumerate(S):
            o = FOFF[im] + (k - RUN[im]) * NV
            dr = p[:, o:o + NV].astype(np.float32)
            o += RL[im] * NV
            di = p[:, o:o + NV].astype(np.float32)
            out[il, im, X, :] = dr + 1j * di
    return out


_NC_CACHE = None


def get_nc():
    global _NC_CACHE
    if _NC_CACHE is None:
        _NC_CACHE = build_bass()
    return _NC_CACHE


def kernel(prev_f_re, prev_f_im, delta0_re, delta0_im, b, v):
    in_maps = pack_inputs(prev_f_re, prev_f_im, delta0_re, delta0_im, b)
    res = run_bass_kernel_spmd(get_nc(), in_maps, list(range(NCORES)))
    return unpack_outputs(res.results, delta0_re, delta0_im)


# revision 16
# speedup vs baseline: 1.2055x; 1.0603x over previous
"""Trainium2 Bass kernel for nn_Bdfdv_51170240364850 (gnn_message_passing).

Computes, for mode pairs (il, im) with im <= il (L1 = 5 modes each way) and
spatial/velocity grid (nx=1024, nv=512):

  D[il,im] = base + (-1j)*im*bx*F[il,im] + cB*bm*F[il,im+1]
             + [im==0] Re(cC*bp*F[il,1])
  base     = 0.5*bm*F[il,im-1]  (il>=1, 1<=im<=il)   else  D0[il,im]

with bx = b[:,0], bm = b[:,1]+1j b[:,2], bp = conj(bm),
cB = -(il-im)(il+im+1)/2, cC = -il(il+1).

Strategy: pure data-parallel over nx across 8 NeuronCores (nx=128 per core on
the 128 SBUF partitions), fp16 I/O, and a three-engine split:

* PE (TensorEngine): every per-x product c(x)*T runs as a diagonal-weight
  matmul accumulating in PSUM (diag(c) @ tile scales partition row p by
  c(p)).  Mode-constant coefficient parts are folded into the operands:
  P = F[im-1] + 2cB*F[im+1] fuses the set & recurrence terms so each output
  column needs only THREE matmuls (0.5b1@P, -+0.5b2@Q, -+im b0@F).  The 12
  tiny diagonal weight tiles (0.5b1, +-0.5b2, +-m*b0, ones) ride in with the
  input DMA.
* DVE: fp16 4x tensor_scalar prescales (P/Q, G = 2cB0*F[1]) plus the im=0
  imaginary row as two fused scalar_tensor_tensor ops (D0i as the add
  operand).
* ACT: evacuates finished PSUM quads (one whole im-run per copy) into the
  fp16 output tile.

PSUM quads ([128, 4 banks] tiles, pool bufs=2) double-buffer the 8 banks;
outputs stream out one im-run at a time.  DMA (~40KB in + 28KB out per
partition, fp16) is the roofline.
"""

import numpy as np

import bass_rust
import concourse.bass as bass
import concourse.tile as tile
from concourse import mybir
from concourse.bass_utils import run_bass_kernel_spmd

L1 = 5
NX = 1024
NV = 512
NCORES = 8
XS = NX // NCORES  # 128, = SBUF partitions

F32 = mybir.dt.float32
F16 = mybir.dt.float16

# ---------------------------------------------------------------------------
# slot bookkeeping (im-major ordering of the 14 valid (im, il>=1) F/D slots)
S = [(im, il) for im in range(L1) for il in range(max(1, im), L1)]
SIDX = {s: k for k, s in enumerate(S)}
NS = len(S)                      # 14
RUN = {0: 0, 1: 4, 2: 8, 3: 11, 4: 13}   # start slot index of each im-run
RL = {0: 4, 1: 4, 2: 3, 3: 2, 4: 1}      # run lengths

CB_PAIRS = [(2, 1), (3, 1), (3, 2), (4, 1), (4, 2), (4, 3)]  # (il, im)

# F/D run-interleaved layout: run m holds [re slots | im slots] back-to-back,
# so each im-run moves as ONE contiguous DMA.
FOFF = {}
_o = 0
for _m in range(L1):
    FOFF[_m] = _o
    _o += 2 * RL[_m] * NV
assert _o == 2 * NS * NV


def _cB(il, im):
    return -(il - im) * (il + im + 1) / 2.0


# pin layout (fp16): [F runs (28 NV) | D0r (4) | D0i (4) | W diags (12x128)]
WOFF = 36 * NV
NDIAG = 13      # 0.5b1, +-0.5b2, A+1..4=m*b0, A-1..4, ones, 1.5b1
DG_D1, DG_D2, DG_D3 = 0, 1, 2
DG_ONES = 11
DG_D6 = 12


def DG_AP(m):
    return 2 + m          # 3..6


def DG_AN(m):
    return 6 + m          # 7..10


CIN = WOFF + NDIAG * 128
# pscal (fp32): per-x scalars for the DVE im=0 imaginary chain
H1, H2 = 0, 1                    # 0.5*b1, 0.5*b2
NSCAL = 4
# pout layout (fp16): same run-interleaved layout as F
COUT = 2 * NS * NV


# ---------------------------------------------------------------------------
# The walrus build in this container rejects instructions carrying more than
# ONE sync-wait ("Too many sync wait commands", setupSyncWait in
# CoreV2/V3GenImpl). Tile's scheduler routinely attaches several. Post-pass:
# hoist all but the last wait of each instruction onto same-engine NOPs
# inserted immediately before it (same basic block, so per-engine program
# order is preserved).
def split_multiwaits(nc):
    for f in nc.m.functions:
        for blk in f.blocks:
            new = []
            changed = False
            for ins in blk.instructions:
                si = ins.sync_info
                if si is not None and len(si.on_wait) > 1:
                    waits = list(si.on_wait)
                    for w in waits[:-1]:
                        nop = mybir.InstNoOp(
                            name=nc.get_next_instruction_name(),
                            engine=ins.engine,
                            bass_nofuse=True,
                            sync_info=mybir.SyncInfo(on_wait=[w],
                                                     on_update=[]),
                        )
                        new.append(nop)
                    ins.sync_info = bass_rust.SyncInfo(
                        on_wait=[waits[-1]], on_update=list(si.on_update))
                    changed = True
                new.append(ins)
            if changed:
                blk.instructions = new


# ---------------------------------------------------------------------------
def _pair(ap, step_elems, nblocks=2):
    """Turn a contiguous [P, L] AP into [P, nblocks, L] with the given
    element step between blocks."""
    c = ap.copy()
    v = c.ap
    last = v.pop()
    v.append((step_elems, nblocks))
    v.append(tuple(last))
    c.ap = v
    return c


def build_bass(split=True):
    MULT = mybir.AluOpType.mult
    ADD = mybir.AluOpType.add

    nc = bass.Bass()
    pin = nc.dram_tensor("pin", [XS, CIN], F16, kind="ExternalInput").ap()
    pscal = nc.dram_tensor("pscal", [XS, NSCAL], F32,
                           kind="ExternalInput").ap()
    pout = nc.dram_tensor("pout", [XS, COUT], F16, kind="ExternalOutput").ap()

    with tile.TileContext(nc) as tc:
        with tc.tile_pool(name="m", bufs=1) as pool, \
             tc.psum_pool(name="p", bufs=4) as ppool:
            fF = pool.tile([XS, 2 * NS * NV], F16, tag="fF")
            fD0 = pool.tile([XS, 8 * NV], F16, tag="fD0")
            fW = pool.tile([XS, NDIAG * 128], F16, tag="fW")
            scal = pool.tile([XS, NSCAL], F32, tag="scal")
            P = pool.tile([XS, 2 * 6 * NV], F16, tag="P")
            G = pool.tile([XS, 2 * 4 * NV], F16, tag="G")
            OUT = pool.tile([XS, 2 * NS * NV], F16, tag="OUT")

            def fslot(k, imag, n=1):
                m = S[k][0]
                o = FOFF[m] + (imag * RL[m] + (k - RUN[m])) * NV
                return fF[:, o:o + n * NV]

            def fr(k):
                return fslot(k, 0)

            def fi(k):
                return fslot(k, 1)

            def pr(j):
                return P[:, j * NV:(j + 1) * NV]

            def pi(j):
                return P[:, (6 + j) * NV:(7 + j) * NV]

            def W(j):
                return fW[:, j * 128:(j + 1) * 128]

            def outr(k, n=1):
                m = S[k][0]
                o = FOFF[m] + (k - RUN[m]) * NV
                return OUT[:, o:o + n * NV]

            def outi(k, n=1):
                m = S[k][0]
                o = FOFF[m] + (RL[m] + k - RUN[m]) * NV
                return OUT[:, o:o + n * NV]

            def sc(col):
                return scal[:, col:col + 1]

            # ---- input DMAs: back-to-back issue; queue FIFO order makes
            # emission order the arrival priority without per-link latency.
            nc.gpsimd.dma_start(scal[:], pscal[:])
            nc.gpsimd.dma_start(fW[:], pin[:, WOFF:WOFF + NDIAG * 128])

            def in_run(m):
                o = FOFF[m]
                n = 2 * RL[m] * NV
                nc.sync.dma_start(fF[:, o:o + n], pin[:, o:o + n])

            in_run(0)
            in_run(1)
            in_run(2)
            nc.sync.dma_start(fD0[:], pin[:, 28 * NV:36 * NV])
            in_run(3)
            in_run(4)

            # ---- DVE prescales ----
            def presc_G(il):        # (Gr,Gi) = 2*cB0(il) * (Fr1,Fi1)
                k1 = SIDX[(1, il)]
                nc.vector.tensor_scalar_mul(
                    _pair(G[:, (il - 1) * NV:il * NV], 4 * NV),
                    _pair(fr(k1), RL[1] * NV),
                    float(-il * (il + 1)))

            def presc_SF(j):        # P = 2cB * F[im+1]   (fp16 TS at 4x)
                il, im = CB_PAIRS[j]
                ks = SIDX[(im + 1, il)]
                nc.vector.tensor_scalar_mul(
                    _pair(pr(j), 6 * NV),
                    _pair(fr(ks), RL[im + 1] * NV),
                    2.0 * _cB(il, im))

            def presc_P(j):         # P += F[im-1]          (fp16 TT at 2x)
                il, im = CB_PAIRS[j]
                kb = SIDX[(im - 1, il)]
                nc.vector.tensor_tensor(
                    _pair(pr(j), 6 * NV),
                    _pair(pr(j), 6 * NV),
                    _pair(fr(kb), RL[im - 1] * NV),
                    ADD)

            # DVE program order = emission order; interleave the im=0
            # imaginary-row chain into the gaps left by input-run arrival.
            gr = G[:, 0:4 * NV]
            gi = G[:, 4 * NV:8 * NV]
            d0i = fD0[:, 4 * NV:8 * NV]
            for j in (0, 1, 3):     # group-1 operands (need runs 0 & 2)
                presc_SF(j)
                presc_P(j)
            for il in range(1, L1):
                presc_G(il)
            for j in (2, 4):        # group-2 (runs 1 & 3)
                presc_SF(j)
                presc_P(j)
            nc.vector.scalar_tensor_tensor(   # Di0 = D0i + 0.5b1*Gi ...
                outi(0, 4), gi, sc(H1), d0i, MULT, ADD)
            presc_SF(5)             # group-3 (runs 2 & 4)
            presc_P(5)
            nc.vector.scalar_tensor_tensor(   # ... + 0.5b2*Gr
                outi(0, 4), gr, sc(H2), outi(0, 4), MULT, ADD)

            # ---- PE groups: quad PSUM tiles, three matmuls per column.
            # diag slot (il==g) reads F directly; middle slots read the
            # merged P/Q operands (set + recurrence fused by the prescale).
            evac = []               # (quad, psum_cols, OUT offset) FIFO

            def mm(bank, j, rhs, start=False, stop=False):
                nc.tensor.matmul(bank, W(j), rhs, start=start, stop=stop,
                                 skip_group_check=True)

            def group(g):
                n = RL[g]
                ils = list(range(g, L1))
                pk = {}
                bR = {}
                bI = {}
                for il in ils:
                    pk[il] = ppool.tile([XS, 2 * NV], F32, tag="pk",
                                        name=f"pk{g}_{il}")
                    bR[il] = pk[il][:, 0:NV]
                    bI[il] = pk[il][:, NV:2 * NV]
                kp = SIDX[(g - 1, g)]
                ks = {il: SIDX[(g, il)] for il in ils}
                jj = {il: CB_PAIRS.index((il, g)) for il in ils if il > g}
                # diag slot: direct F sources (earliest-arriving runs)
                mm(bR[g], DG_D1, fr(kp), start=True)
                mm(bI[g], DG_D1, fi(kp), start=True)
                mm(bR[g], DG_D3, fi(kp))
                mm(bI[g], DG_D2, fr(kp))
                # cA pass opens the middle banks, closes the diag bank
                for il in ils:
                    mm(bR[il], DG_AP(g), fi(ks[il]), start=(il > g),
                       stop=(il == g))
                for il in ils:
                    mm(bI[il], DG_AN(g), fr(ks[il]), start=(il > g),
                       stop=(il == g))
                # merged set+cB products close the middle banks
                for il in ils:
                    if il > g:
                        mm(bR[il], DG_D1, pr(jj[il]))
                        mm(bI[il], DG_D1, pi(jj[il]))
                for il in ils:
                    if il > g:
                        mm(bR[il], DG_D3, pi(jj[il]), stop=True)
                        mm(bI[il], DG_D2, pr(jj[il]), stop=True)
                for il in ils:
                    evac.append((pk[il][:],
                                 _pair(outr(ks[il]), RL[g] * NV)))

            def flush_evac():
                while evac:
                    src_, dst = evac.pop(0)
                    nc.scalar.copy(dst, src_)

            def out_run(m):
                o = FOFF[m]
                n = 2 * RL[m] * NV
                nc.sync.dma_start(pout[:, o:o + n], OUT[:, o:o + n])

            group(1)
            flush_evac()
            group(2)
            out_run(1)
            flush_evac()

            # im0 real row on PE: Dr0 = 1.5b1@Gr + 0.5b2@Gi + ones@D0r
            b0p = {0: ppool.tile([XS, 2 * NV], F32, tag="pk", name="b0a"),
                   1: ppool.tile([XS, 2 * NV], F32, tag="pk", name="b0b")}

            def b0k(il):
                return b0p[(il - 1) // 2][:, ((il - 1) % 2) * NV:
                                          ((il - 1) % 2 + 1) * NV]

            for il in range(1, L1):
                mm(b0k(il), DG_D6, G[:, (il - 1) * NV:il * NV], start=True)
            for il in range(1, L1):
                mm(b0k(il), DG_D2, G[:, (3 + il) * NV:(4 + il) * NV])
            for il in range(1, L1):
                mm(b0k(il), DG_ONES, fD0[:, (il - 1) * NV:il * NV],
                   stop=True)
            out_run(2)

            group(3)
            evac.insert(0, (b0p[0][:], OUT[:, 0:2 * NV]))
            evac.insert(1, (b0p[1][:], OUT[:, 2 * NV:4 * NV]))
            flush_evac()
            group(4)
            out_run(3)
            flush_evac()
            out_run(0)
            out_run(4)

    if split:
        split_multiwaits(nc)
    return nc


# ---------------------------------------------------------------------------
def pack_inputs(prev_f_re, prev_f_im, delta0_re, delta0_im, b):
    """-> list of per-core {'pin': [XS, CIN] f16, 'pscal': [XS, 4] f32}."""
    pr = np.asarray(prev_f_re, np.float32)
    pi = np.asarray(prev_f_im, np.float32)
    d0r = np.asarray(delta0_re, np.float32)
    d0i = np.asarray(delta0_im, np.float32)
    bb = np.asarray(b, np.float32)
    ar = np.arange(XS)
    in_maps = []
    for c in range(NCORES):
        X = slice(c * XS, (c + 1) * XS)
        p = np.zeros((XS, CIN), np.float16)
        for k, (im, il) in enumerate(S):
            o = FOFF[im] + (k - RUN[im]) * NV
            p[:, o:o + NV] = pr[il, im, X, :]
            o += RL[im] * NV
            p[:, o:o + NV] = pi[il, im, X, :]
        for il in range(1, L1):
            p[:, (28 + il - 1) * NV:(29 + il - 1) * NV] = d0r[il, 0, X, :]
            p[:, (32 + il - 1) * NV:(33 + il - 1) * NV] = d0i[il, 0, X, :]
        b0, b1, b2 = bb[X, 0], bb[X, 1], bb[X, 2]
        diags = [0.5 * b1, 0.5 * b2, -0.5 * b2,
                 1.0 * b0, 2.0 * b0, 3.0 * b0, 4.0 * b0,
                 -1.0 * b0, -2.0 * b0, -3.0 * b0, -4.0 * b0,
                 np.ones(XS, np.float32), 1.5 * b1]
        for j, cx in enumerate(diags):
            blk = np.zeros((XS, 128), np.float16)
            blk[ar, ar] = cx.astype(np.float16)
            p[:, WOFF + j * 128:WOFF + (j + 1) * 128] = blk
        ps = np.zeros((XS, NSCAL), np.float32)
        ps[:, H1] = 0.5 * b1
        ps[:, H2] = 0.5 * b2
        in_maps.append({"pin": p, "pscal": ps})
    return in_maps


def unpack_outputs(results, delta0_re, delta0_im):
    out = np.zeros((L1, L1, NX, NV), np.complex64)
    out[0, 0] = np.asarray(delta0_re[0, 0]) + 1j * np.asarray(delta0_im[0, 0])
    for c in range(NCORES):
        X = slice(c * XS, (c + 1) * XS)
        p = results[c]["pout"]
        for k, (im, il) in enumerate(S):
            o = FOFF[im] + (k - RUN[im]) * NV
            dr = p[:, o:o + NV].astype(np.float32)
            o += RL[im] * NV
            di = p[:, o:o + NV].astype(np.float32)
            out[il, im, X, :] = dr + 1j * di
    return out


_NC_CACHE = None


def get_nc():
    global _NC_CACHE
    if _NC_CACHE is None:
        _NC_CACHE = build_bass()
    return _NC_CACHE


def kernel(prev_f_re, prev_f_im, delta0_re, delta0_im, b, v):
    in_maps = pack_inputs(prev_f_re, prev_f_im, delta0_re, delta0_im, b)
    res = run_bass_kernel_spmd(get_nc(), in_maps, list(range(NCORES)))
    return unpack_outputs(res.results, delta0_re, delta0_im)


# revision 18
# speedup vs baseline: 1.2359x; 1.0252x over previous
"""Trainium2 Bass kernel for nn_Bdfdv_51170240364850 (gnn_message_passing).

Computes, for mode pairs (il, im) with im <= il (L1 = 5 modes each way) and
spatial/velocity grid (nx=1024, nv=512):

  D[il,im] = base + (-1j)*im*bx*F[il,im] + cB*bm*F[il,im+1]
             + [im==0] Re(cC*bp*F[il,1])
  base     = 0.5*bm*F[il,im-1]  (il>=1, 1<=im<=il)   else  D0[il,im]

with bx = b[:,0], bm = b[:,1]+1j b[:,2], bp = conj(bm),
cB = -(il-im)(il+im+1)/2, cC = -il(il+1).

Strategy: pure data-parallel over nx across 8 NeuronCores (nx=128 per core on
the 128 SBUF partitions), fp16 I/O, and a three-engine split:

* PE (TensorEngine): every per-x product c(x)*T runs as a diagonal-weight
  matmul accumulating in PSUM (diag(c) @ tile scales partition row p by
  c(p)).  Mode-constant coefficient parts are folded into the operands:
  P = F[im-1] + 2cB*F[im+1] fuses the set & recurrence terms so each output
  column needs only THREE matmuls (0.5b1@P, -+0.5b2@Q, -+im b0@F).  The 12
  tiny diagonal weight tiles (0.5b1, +-0.5b2, +-m*b0, ones) ride in with the
  input DMA.
* DVE: fp16 4x tensor_scalar prescales (P/Q, G = 2cB0*F[1]) plus the im=0
  imaginary row as two fused scalar_tensor_tensor ops (D0i as the add
  operand).
* ACT: evacuates finished PSUM quads (one whole im-run per copy) into the
  fp16 output tile.

PSUM quads ([128, 4 banks] tiles, pool bufs=2) double-buffer the 8 banks;
outputs stream out one im-run at a time.  DMA (~40KB in + 28KB out per
partition, fp16) is the roofline.
"""

import numpy as np

import bass_rust
import concourse.bass as bass
import concourse.tile as tile
from concourse import mybir
from concourse.bass_utils import run_bass_kernel_spmd

L1 = 5
NX = 1024
NV = 512
NCORES = 8
XS = NX // NCORES  # 128, = SBUF partitions

F32 = mybir.dt.float32
F16 = mybir.dt.float16

# ---------------------------------------------------------------------------
# slot bookkeeping (im-major ordering of the 14 valid (im, il>=1) F/D slots)
S = [(im, il) for im in range(L1) for il in range(max(1, im), L1)]
SIDX = {s: k for k, s in enumerate(S)}
NS = len(S)                      # 14
RUN = {0: 0, 1: 4, 2: 8, 3: 11, 4: 13}   # start slot index of each im-run
RL = {0: 4, 1: 4, 2: 3, 3: 2, 4: 1}      # run lengths

CB_PAIRS = [(2, 1), (3, 1), (3, 2), (4, 1), (4, 2), (4, 3)]  # (il, im)

# F/D run-interleaved layout: run m holds [re slots | im slots] back-to-back,
# so each im-run moves as ONE contiguous DMA.
FOFF = {}
_o = 0
for _m in range(L1):
    FOFF[_m] = _o
    _o += 2 * RL[_m] * NV
assert _o == 2 * NS * NV


def _cB(il, im):
    return -(il - im) * (il + im + 1) / 2.0


# pin layout (fp16): [F runs (28 NV) | D0r (4) | D0i (4) | W diags (12x128)]
WOFF = 36 * NV
NDIAG = 13      # 0.5b1, +-0.5b2, A+1..4=m*b0, A-1..4, ones, 1.5b1
DG_D1, DG_D2, DG_D3 = 0, 1, 2
DG_ONES = 11
DG_D6 = 12


def DG_AP(m):
    return 2 + m          # 3..6


def DG_AN(m):
    return 6 + m          # 7..10


CIN = WOFF + NDIAG * 128
# pscal (fp32): per-x scalars for the DVE im=0 imaginary chain
H1, H2 = 0, 1                    # 0.5*b1, 0.5*b2
NSCAL = 4
# pout layout (fp16): same run-interleaved layout as F
COUT = 2 * NS * NV


# ---------------------------------------------------------------------------
# The walrus build in this container rejects instructions carrying more than
# ONE sync-wait ("Too many sync wait commands", setupSyncWait in
# CoreV2/V3GenImpl). Tile's scheduler routinely attaches several. Post-pass:
# hoist all but the last wait of each instruction onto same-engine NOPs
# inserted immediately before it (same basic block, so per-engine program
# order is preserved).
def split_multiwaits(nc):
    for f in nc.m.functions:
        for blk in f.blocks:
            new = []
            changed = False
            for ins in blk.instructions:
                si = ins.sync_info
                if si is not None and len(si.on_wait) > 1:
                    waits = list(si.on_wait)
                    for w in waits[:-1]:
                        nop = mybir.InstNoOp(
                            name=nc.get_next_instruction_name(),
                            engine=ins.engine,
                            bass_nofuse=True,
                            sync_info=mybir.SyncInfo(on_wait=[w],
                                                     on_update=[]),
                        )
                        new.append(nop)
                    ins.sync_info = bass_rust.SyncInfo(
                        on_wait=[waits[-1]], on_update=list(si.on_update))
                    changed = True
                new.append(ins)
            if changed:
                blk.instructions = new


# ---------------------------------------------------------------------------
def _pair(ap, step_elems, nblocks=2):
    """Turn a contiguous [P, L] AP into [P, nblocks, L] with the given
    element step between blocks."""
    c = ap.copy()
    v = c.ap
    last = v.pop()
    v.append((step_elems, nblocks))
    v.append(tuple(last))
    c.ap = v
    return c


def build_bass(split=True):
    MULT = mybir.AluOpType.mult
    ADD = mybir.AluOpType.add

    nc = bass.Bass()
    pin = nc.dram_tensor("pin", [XS, CIN], F16, kind="ExternalInput").ap()
    pscal = nc.dram_tensor("pscal", [XS, NSCAL], F32,
                           kind="ExternalInput").ap()
    pout = nc.dram_tensor("pout", [XS, COUT], F16, kind="ExternalOutput").ap()

    with tile.TileContext(nc) as tc:
        with tc.tile_pool(name="m", bufs=1) as pool, \
             tc.psum_pool(name="p", bufs=4) as ppool:
            fF = pool.tile([XS, 2 * NS * NV], F16, tag="fF")
            fD0 = pool.tile([XS, 8 * NV], F16, tag="fD0")
            fW = pool.tile([XS, NDIAG * 128], F16, tag="fW")
            scal = pool.tile([XS, NSCAL], F32, tag="scal")
            P = pool.tile([XS, 2 * 6 * NV], F16, tag="P")
            G = pool.tile([XS, 2 * 4 * NV], F16, tag="G")
            OUT = pool.tile([XS, 2 * NS * NV], F16, tag="OUT")

            def fslot(k, imag, n=1):
                m = S[k][0]
                o = FOFF[m] + (imag * RL[m] + (k - RUN[m])) * NV
                return fF[:, o:o + n * NV]

            def fr(k):
                return fslot(k, 0)

            def fi(k):
                return fslot(k, 1)

            def pr(j):
                return P[:, j * NV:(j + 1) * NV]

            def pi(j):
                return P[:, (6 + j) * NV:(7 + j) * NV]

            def W(j):
                return fW[:, j * 128:(j + 1) * 128]

            def outr(k, n=1):
                m = S[k][0]
                o = FOFF[m] + (k - RUN[m]) * NV
                return OUT[:, o:o + n * NV]

            def outi(k, n=1):
                m = S[k][0]
                o = FOFF[m] + (RL[m] + k - RUN[m]) * NV
                return OUT[:, o:o + n * NV]

            def sc(col):
                return scal[:, col:col + 1]

            # ---- input DMAs: all issued from the GpSimd queue, which is
            # live ~6us before the sync sequencer finishes its preamble;
            # FIFO drain makes emission order the arrival priority.
            nc.gpsimd.dma_start(scal[:], pscal[:])
            nc.gpsimd.dma_start(fW[:], pin[:, WOFF:WOFF + NDIAG * 128])

            def in_run(m):
                o = FOFF[m]
                n = 2 * RL[m] * NV
                nc.gpsimd.dma_start(fF[:, o:o + n], pin[:, o:o + n])

            in_run(0)
            in_run(1)
            in_run(2)
            nc.gpsimd.dma_start(fD0[:], pin[:, 28 * NV:36 * NV])
            in_run(3)
            in_run(4)

            # ---- DVE prescales ----
            def presc_G(il):        # (Gr,Gi) = 2*cB0(il) * (Fr1,Fi1)
                k1 = SIDX[(1, il)]
                nc.vector.tensor_scalar_mul(
                    _pair(G[:, (il - 1) * NV:il * NV], 4 * NV),
                    _pair(fr(k1), RL[1] * NV),
                    float(-il * (il + 1)))

            def presc_SF(j):        # P = 2cB * F[im+1]   (fp16 TS at 4x)
                il, im = CB_PAIRS[j]
                ks = SIDX[(im + 1, il)]
                nc.vector.tensor_scalar_mul(
                    _pair(pr(j), 6 * NV),
                    _pair(fr(ks), RL[im + 1] * NV),
                    2.0 * _cB(il, im))

            def presc_P(j):         # P += F[im-1]          (fp16 TT at 2x)
                il, im = CB_PAIRS[j]
                kb = SIDX[(im - 1, il)]
                nc.vector.tensor_tensor(
                    _pair(pr(j), 6 * NV),
                    _pair(pr(j), 6 * NV),
                    _pair(fr(kb), RL[im - 1] * NV),
                    ADD)

            # DVE program order = emission order; interleave the im=0
            # imaginary-row chain into the gaps left by input-run arrival.
            gr = G[:, 0:4 * NV]
            gi = G[:, 4 * NV:8 * NV]
            d0i = fD0[:, 4 * NV:8 * NV]
            for j in (0, 1, 3):     # group-1 operands (need runs 0 & 2)
                presc_SF(j)
                presc_P(j)
            for il in range(1, L1):
                presc_G(il)
            for j in (2, 4):        # group-2 (runs 1 & 3)
                presc_SF(j)
                presc_P(j)
            nc.vector.scalar_tensor_tensor(   # Di0 = D0i + 0.5b1*Gi ...
                outi(0, 4), gi, sc(H1), d0i, MULT, ADD)
            presc_SF(5)             # group-3 (runs 2 & 4)
            presc_P(5)
            nc.vector.scalar_tensor_tensor(   # ... + 0.5b2*Gr
                outi(0, 4), gr, sc(H2), outi(0, 4), MULT, ADD)

            # ---- PE groups: quad PSUM tiles, three matmuls per column.
            # diag slot (il==g) reads F directly; middle slots read the
            # merged P/Q operands (set + recurrence fused by the prescale).
            evac = []               # (quad, psum_cols, OUT offset) FIFO

            def mm(bank, j, rhs, start=False, stop=False):
                nc.tensor.matmul(bank, W(j), rhs, start=start, stop=stop,
                                 skip_group_check=True)

            def group(g):
                n = RL[g]
                ils = list(range(g, L1))
                pk = {}
                bR = {}
                bI = {}
                for il in ils:
                    pk[il] = ppool.tile([XS, 2 * NV], F32, tag="pk",
                                        name=f"pk{g}_{il}")
                    bR[il] = pk[il][:, 0:NV]
                    bI[il] = pk[il][:, NV:2 * NV]
                kp = SIDX[(g - 1, g)]
                ks = {il: SIDX[(g, il)] for il in ils}
                jj = {il: CB_PAIRS.index((il, g)) for il in ils if il > g}
                # diag slot: direct F sources (earliest-arriving runs)
                mm(bR[g], DG_D1, fr(kp), start=True)
                mm(bI[g], DG_D1, fi(kp), start=True)
                mm(bR[g], DG_D3, fi(kp))
                mm(bI[g], DG_D2, fr(kp))
                # cA pass opens the middle banks, closes the diag bank
                for il in ils:
                    mm(bR[il], DG_AP(g), fi(ks[il]), start=(il > g),
                       stop=(il == g))
                for il in ils:
                    mm(bI[il], DG_AN(g), fr(ks[il]), start=(il > g),
                       stop=(il == g))
                # merged set+cB products close the middle banks
                for il in ils:
                    if il > g:
                        mm(bR[il], DG_D1, pr(jj[il]))
                        mm(bI[il], DG_D1, pi(jj[il]))
                for il in ils:
                    if il > g:
                        mm(bR[il], DG_D3, pi(jj[il]), stop=True)
                        mm(bI[il], DG_D2, pr(jj[il]), stop=True)
                for il in ils:
                    evac.append((pk[il][:],
                                 _pair(outr(ks[il]), RL[g] * NV)))

            def flush_evac(engine="act"):
                while evac:
                    src_, dst = evac.pop(0)
                    if engine == "act":
                        nc.scalar.copy(dst, src_)
                    else:
                        nc.vector.tensor_copy(dst, src_)

            def out_run(m):
                o = FOFF[m]
                n = 2 * RL[m] * NV
                nc.sync.dma_start(pout[:, o:o + n], OUT[:, o:o + n])

            group(1)
            flush_evac()
            group(2)
            out_run(1)
            flush_evac()

            # im0 real row on PE: Dr0 = 1.5b1@Gr + 0.5b2@Gi + ones@D0r
            b0p = {0: ppool.tile([XS, 2 * NV], F32, tag="pk", name="b0a"),
                   1: ppool.tile([XS, 2 * NV], F32, tag="pk", name="b0b")}

            def b0k(il):
                return b0p[(il - 1) // 2][:, ((il - 1) % 2) * NV:
                                          ((il - 1) % 2 + 1) * NV]

            for il in range(1, L1):
                mm(b0k(il), DG_D6, G[:, (il - 1) * NV:il * NV], start=True)
            for il in range(1, L1):
                mm(b0k(il), DG_D2, G[:, (3 + il) * NV:(4 + il) * NV])
            for il in range(1, L1):
                mm(b0k(il), DG_ONES, fD0[:, (il - 1) * NV:il * NV],
                   stop=True)
            out_run(2)

            group(3)
            # b0 pairs drain on ACT; group-3/4 pairs on DVE (idle by then)
            nc.scalar.copy(OUT[:, 0:2 * NV], b0p[0][:])
            nc.scalar.copy(OUT[:, 2 * NV:4 * NV], b0p[1][:])
            flush_evac("dve")
            group(4)
            out_run(3)
            flush_evac("dve")
            out_run(0)
            out_run(4)

    if split:
        split_multiwaits(nc)
    return nc


# ---------------------------------------------------------------------------
def pack_inputs(prev_f_re, prev_f_im, delta0_re, delta0_im, b):
    """-> list of per-core {'pin': [XS, CIN] f16, 'pscal': [XS, 4] f32}."""
    pr = np.asarray(prev_f_re, np.float32)
    pi = np.asarray(prev_f_im, np.float32)
    d0r = np.asarray(delta0_re, np.float32)
    d0i = np.asarray(delta0_im, np.float32)
    bb = np.asarray(b, np.float32)
    ar = np.arange(XS)
    in_maps = []
    for c in range(NCORES):
        X = slice(c * XS, (c + 1) * XS)
        p = np.zeros((XS, CIN), np.float16)
        for k, (im, il) in enumerate(S):
            o = FOFF[im] + (k - RUN[im]) * NV
            p[:, o:o + NV] = pr[il, im, X, :]
            o += RL[im] * NV
            p[:, o:o + NV] = pi[il, im, X, :]
        for il in range(1, L1):
            p[:, (28 + il - 1) * NV:(29 + il - 1) * NV] = d0r[il, 0, X, :]
            p[:, (32 + il - 1) * NV:(33 + il - 1) * NV] = d0i[il, 0, X, :]
        b0, b1, b2 = bb[X, 0], bb[X, 1], bb[X, 2]
        diags = [0.5 * b1, 0.5 * b2, -0.5 * b2,
                 1.0 * b0, 2.0 * b0, 3.0 * b0, 4.0 * b0,
                 -1.0 * b0, -2.0 * b0, -3.0 * b0, -4.0 * b0,
                 np.ones(XS, np.float32), 1.5 * b1]
        for j, cx in enumerate(diags):
            blk = np.zeros((XS, 128), np.float16)
            blk[ar, ar] = cx.astype(np.float16)
            p[:, WOFF + j * 128:WOFF + (j + 1) * 128] = blk
        ps = np.zeros((XS, NSCAL), np.float32)
        ps[:, H1] = 0.5 * b1
        ps[:, H2] = 0.5 * b2
        in_maps.append({"pin": p, "pscal": ps})
    return in_maps


def unpack_outputs(results, delta0_re, delta0_im):
    out = np.zeros((L1, L1, NX, NV), np.complex64)
    out[0, 0] = np.asarray(delta0_re[0, 0]) + 1j * np.asarray(delta0_im[0, 0])
    for c in range(NCORES):
        X = slice(c * XS, (c + 1) * XS)
        p = results[c]["pout"]
        for k, (im, il) in enumerate(S):
            o = FOFF[im] + (k - RUN[im]) * NV
            dr = p[:, o:o + NV].astype(np.float32)
            o += RL[im] * NV
            di = p[:, o:o + NV].astype(np.float32)
            out[il, im, X, :] = dr + 1j * di
    return out


_NC_CACHE = None


def get_nc():
    global _NC_CACHE
    if _NC_CACHE is None:
        _NC_CACHE = build_bass()
    return _NC_CACHE


def kernel(prev_f_re, prev_f_im, delta0_re, delta0_im, b, v):
    in_maps = pack_inputs(prev_f_re, prev_f_im, delta0_re, delta0_im, b)
    res = run_bass_kernel_spmd(get_nc(), in_maps, list(range(NCORES)))
    return unpack_outputs(res.results, delta0_re, delta0_im)
